# revision 36
# baseline (speedup 1.0000x reference)
"""GCN (2x GCNConv + classifier + log_softmax) on 8 Trainium2 NeuronCores.

Strategy (node sharding, per-core c owns rows [c*NPC, (c+1)*NPC)):
  h  = dinv * (x @ W1)      dense per-core rows from host-pre-transposed xT
                            (bf16, streamed in chunks); the own-node degree
                            norm is folded into the PSUM->fp8 cast on the
                            Scalar engine. h rows are staged to DRAM per
                            block and AllGathered in S=3 pipelined chunks
                            (fired as soon as each chunk's blocks are done).
  agg1 = Ahat @ h           dma_gather rows (one call per (dst-block, src-
                            stream)) + one-hot matmul into PSUM per 128-dst
                            block; the one-hot M is generated ON-CHIP per
                            block by a single DVE is_equal against a resident
                            iota table. Flush = psum * dinv[dst] (DVE) then
                            h1T = Relu(+b1) on ScalarE, in transposed layout
                            (features on partitions).
  z2 = dinv * (h1 @ W2)     dense from h1T (Scalar Copy applies dinv);
                            staged + AllGathered in 3 chunks like h.
  agg2 = Ahat @ z2          same SpMM -> h2T = Relu(psum*dinv[dst] + b2) on
                            DVE (tensor_tensor add/max; Scalar stays on the
                            Exp table).
  logits = h2 @ Wc + bc     per dst block right after its SpMM flush: add bc,
                            exp, row-sum. The Ln + subtract + output DMA run
                            batched after SpMM2 (one activation-table load).

Edge handling (host planner): self-loops become virtual edges; edges are
partitioned by dst-owner core, split into S=3 streams by src chunk (so
int16 gather indices reach, and so each chunk's AllGather pipelines with
compute), grouped by 128-node dst block. Per (block, stream) one gather
call per 8 tiles: num_idxs_reg = max real edge count over cores; each core
pads its index list with 0 (valid row) and the one-hot has zero columns for
pad slots.
"""
import sys
import numpy as np

sys.path.insert(0, '/opt/trn_rl_repo')

import ml_dtypes

BF16NP = ml_dtypes.bfloat16

N, E_EXPECT, IN, HID, MID, OUT = 50000, 800000, 512, 256, 128, 64
NC = 8
TILE = 128           # edges per matmul tile (contraction dim)
BLK = 128            # dst nodes per PSUM accumulation block
NSTREAM = 3


def _derived(n=None, nc=NC):
    if n is None:
        n = N
    npc = n // nc
    nb = (npc + 127) // 128
    # stream boundaries in 128-row blocks; per-stream rows*nc must stay
    # within int16 gather-index reach (nc*rows <= 32768). Boundaries are
    # multiples of 7 (the xT/staging chunk) and the LAST stream is small so
    # the final AllGather (which gates each SpMM's tail stream) ends early.
    maxb = 32768 // (128 * nc)
    b1 = min(maxb, max(7, (nb // 7 // 2) * 7))
    b2 = min(b1 + maxb, max(b1 + 7, ((nb - 7) // 7) * 7))
    bounds = [0, min(b1, nb), min(b2, nb), nb]
    bounds = sorted(set(bounds))
    while len(bounds) - 1 < NSTREAM:
        bounds.append(nb)
        bounds = sorted(set(bounds))
    rpc = []
    for s in range(len(bounds) - 1):
        hi = bounds[s + 1] * 128 if s < len(bounds) - 2 else npc
        rpc.append(hi - bounds[s] * 128)
    return dict(NPC=npc, NB128=nb, BOUNDS=bounds, RPC=rpc,
                ROW0=[b * 128 for b in bounds[:-1]])


# ---------------------------------------------------------------- host planner

def plan(edge_index: np.ndarray, n=None, nc=NC):
    if n is None:
        n = N
    d = _derived(n, nc)
    npc = d['NPC']
    nblk = d['NB128']
    rpc, row0 = d['RPC'], d['ROW0']
    S = len(rpc)

    src = edge_index[0].astype(np.int64)
    dst = edge_index[1].astype(np.int64)
    deg = np.ones(n, np.float64)
    np.add.at(deg, dst, 1.0)
    dinv = (1.0 / np.sqrt(deg)).astype(np.float32)

    # balanced dst->(core, position) assignment: deal nodes in degree order
    # into per-block bands (8*128 nodes), then greedily pick each node's core
    # within its band to flatten per-(core, stream, block) edge counts, since
    # the schedule pads every (stream, block) group to the max over cores.
    order_by_deg = np.argsort(-deg, kind='stable')
    i_all = np.arange(n, dtype=np.int64)
    apos_prov = np.empty(n, np.int64)
    apos_prov[order_by_deg] = i_all // nc      # band/block is fixed by rank

    allsrc = np.concatenate([src, np.arange(n, dtype=np.int64)])
    alldst = np.concatenate([dst, np.arange(n, dtype=np.int64)])

    rowb_prov = np.array([0] + list(np.cumsum(rpc)), np.int64)
    sof_node = (np.searchsorted(rowb_prov, apos_prov, side='right') - 1)
    sv = np.zeros((n, S), np.int64)            # per-node in-edges by src stream
    np.add.at(sv, (alldst, sof_node[allsrc]), 1)

    acore = np.empty(n, np.int64)
    apos = np.empty(n, np.int64)
    for band in range(nblk):
        r0 = band * nc * 128
        r1 = min(r0 + nc * 128, n)
        nodes = order_by_deg[r0:r1]
        cap = (r1 - r0) // nc
        load = np.zeros((nc, S), np.int64)
        ncnt = np.zeros(nc, np.int64)
        # heaviest nodes first; place on the core that keeps the worst
        # (core, stream) load smallest
        bands_sv = sv[nodes]
        order2 = np.argsort(-bands_sv.sum(axis=1), kind='stable')
        for j in order2:
            v = nodes[j]
            dv_ = bands_sv[j]
            colmax = load.max(axis=0)
            best, bkey = -1, None
            for c in range(nc):
                if ncnt[c] >= cap:
                    continue
                cand = load[c] + dv_
                # actual padding objective: sum over streams of the
                # per-stream max-over-cores this placement would produce
                key = (int(np.maximum(colmax, cand).sum()), int(cand.sum()))
                if best < 0 or key < bkey:
                    best, bkey = c, key
            load[best] += dv_
            acore[v] = best
            apos[v] = band * 128 + ncnt[best]
            ncnt[best] += 1
    nodeof = np.empty(n, np.int64)
    nodeof[acore * npc + apos] = np.arange(n, dtype=np.int64)

    core = acore[alldst]
    scor = acore[allsrc]
    srow = apos[allsrc]
    rowb = np.array([0] + list(np.cumsum(rpc)), np.int64)
    sof = np.searchsorted(rowb, srow, side='right') - 1      # stream id
    sidx = scor * np.array(rpc)[sof] + (srow - rowb[sof])    # idx within h_s
    block = apos[alldst] // BLK
    dloc = apos[alldst] % BLK

    counts = np.zeros((nc, S, nblk), np.int64)
    np.add.at(counts, (core, sof, block), 1)
    cnt_bs = counts.max(axis=0)            # [S, nblk] max real count over cores
    sched = -(-cnt_bs // TILE)             # [S, nblk] tiles per (stream, block)

    # per-core edge lists grouped by (stream, block), in slot order
    per_core = []
    for c in range(nc):
        m = core == c
        csrc, cst, cblock, cdloc = sidx[m], sof[m], block[m], dloc[m]
        stream_idx = {}
        per_s_dl = {}
        for h in range(S):
            mh = cst == h
            hsrc = csrc[mh]
            hblock, hdloc = cblock[mh], cdloc[mh]
            order = np.argsort(hblock, kind='stable')
            hsrc, hblock, hdloc = hsrc[order], hblock[order], hdloc[order]
            bs = np.searchsorted(hblock, np.arange(nblk))
            be = np.searchsorted(hblock, np.arange(nblk), side='right')
            idxs, dls = [], []
            for bb in range(nblk):
                nslots = sched[h, bb] * TILE
                ne = be[bb] - bs[bb]
                assert ne <= nslots
                sl = slice(bs[bb], be[bb])
                ii = np.zeros(nslots, np.int64)    # pads gather row 0 (valid)
                dd = np.full(nslots, -1, np.int64)
                o2 = np.argsort(hsrc[sl], kind='stable')   # HBM locality
                ii[:ne] = hsrc[sl][o2]
                dd[:ne] = hdloc[sl][o2]
                idxs.append(ii)
                dls.append(dd)
            flat = np.concatenate(idxs) if idxs else np.zeros(0, np.int64)
            ntl = len(flat) // TILE
            padt = (-ntl) % 8                  # pad to full 8-tile gather calls
            if padt:
                flat = np.concatenate([flat, np.zeros(padt * TILE, np.int64)])
            stream_idx[h] = flat
            per_s_dl[h] = dls

        # consumption order (per block: stream 0..S-1 tiles) for dl
        dl_cons = []
        for bb in range(nblk):
            for h in range(S):
                dl_cons.append(per_s_dl[h][bb])
        dl_cons = np.concatenate(dl_cons)

        def gather_layout(flat_idx, call_tiles=8):
            """[T*128] int -> [128, T*8] int16 in dma_gather layout,
            call-blocked every call_tiles tiles."""
            ntiles = len(flat_idx) // TILE
            cols = np.zeros((16, ntiles * 8), np.int16)
            pos = 0
            for c0 in range(0, ntiles, call_tiles):
                nt = min(call_tiles, ntiles - c0)
                nidx = nt * TILE
                chunk = flat_idx[pos:pos + nidx].astype(np.int16)
                cols[:, c0 * 8:c0 * 8 + nt * 8] = chunk.reshape(-1, 16).T
                pos += nidx
            return np.tile(cols, (8, 1))

        # dl as bf16 [128, NT] in slot layout: slot e of tile t ->
        # partition e, col t
        ntt = len(dl_cons) // TILE
        dlt = dl_cons.reshape(ntt, TILE).T.astype(np.float32)   # [128, NT]
        per_core.append(dict(
            idx=[gather_layout(stream_idx[h]) for h in range(S)],
            dl=dlt.astype(BF16NP)))

    return dict(sched=[list(sched[s]) for s in range(S)],
                per_core=per_core, dinv=dinv, d=d, nodeof=nodeof)


# ---------------------------------------------------------------- bass program

def build(nc_obj, p, n=None, ncores=NC):
    if n is None:
        n = N
    from concourse import bass, mybir, tile

    BF = mybir.dt.bfloat16
    F32 = mybir.dt.float32
    I16 = mybir.dt.int16
    FP8 = mybir.dt.float8e4

    sched = p['sched']
    d = _derived(n, ncores)
    NPC, NB128 = d['NPC'], d['NB128']
    BOUNDS, RPC = d['BOUNDS'], d['RPC']
    S = len(RPC)
    RTOT = [r * ncores for r in RPC]
    NBLK = NB128
    TS = [sum(sched[s]) + (-sum(sched[s])) % 8 for s in range(S)]
    NT = sum(sum(sched[s]) for s in range(S))
    MAXBT = max(sum(sched[s][b] for s in range(S)) for b in range(NBLK))
    NFULL = NPC // 128            # full 128-row blocks
    NREM = NPC - NFULL * 128      # rows in the final partial block

    b = nc_obj
    xT_d = b.declare_dram_parameter("xT", [IN, NB128 * 128], BF, isOutput=False)
    W1_d = b.declare_dram_parameter("W1", [IN, HID], BF, isOutput=False)
    W2_d = b.declare_dram_parameter("W2", [HID, MID], BF, isOutput=False)
    Wc_d = b.declare_dram_parameter("Wc", [MID, OUT], BF, isOutput=False)
    b1_d = b.declare_dram_parameter("b1c", [128, HID // 128], F32, isOutput=False)
    b2_d = b.declare_dram_parameter("b2c", [128, MID // 128], F32, isOutput=False)
    bc_d = b.declare_dram_parameter("bcr", [128, OUT], F32, isOutput=False)
    ix_d = [b.declare_dram_parameter(f"idx{s}", [128, TS[s] * 8], I16,
                                     isOutput=False) for s in range(S)]
    dl_d = b.declare_dram_parameter("dl", [128, NT], BF, isOutput=False)
    io_d = b.declare_dram_parameter("iota", [128, MAXBT * BLK], BF, isOutput=False)
    dvo_d = b.declare_dram_parameter("dvo", [128, NB128], F32, isOutput=False)
    dvd_d = b.declare_dram_parameter("dvd", [128, NBLK * BLK], BF, isOutput=False)
    out_d = b.declare_dram_parameter("out", [NPC, OUT], F32, isOutput=True)

    with tile.TileContext(b) as tc:
        nreg_cm = b.gpsimd.register("nidx")
        nreg = nreg_cm.__enter__()
        b.gpsimd.reg_mov(nreg, 8 * TILE)
        # ---- resident SBUF tables
        W1_s, W1_s_free = tc.tile([128, IN // 128, HID], BF, name="W1s")
        W2_s, W2_s_free = tc.tile([128, HID // 128, MID], BF, name="W2s")
        Wc_s, Wc_s_free = tc.tile([128, OUT], BF, name="Wcs")
        b1_s, b1_s_free = tc.tile([128, HID // 128], F32, name="b1s")
        b2_s, b2_s_free = tc.tile([128, MID // 128], F32, name="b2s")
        bc_s, bc_s_free = tc.tile([128, OUT], F32, name="bcs")
        ix_s = []
        ix_free = []
        for s in range(S):
            t, f = tc.tile([128, TS[s] * 8], I16, name=f"ix{s}")
            ix_s.append(t)
            ix_free.append(f)
        dl_s, dl_s_free = tc.tile([128, NT], BF, name="dls")
        io_s, io_s_free = tc.tile([128, MAXBT, BLK], BF, name="ios")
        dvo_s, dvo_s_free = tc.tile([128, NB128], F32, name="dvos")
        dvd_s, dvd_s_free = tc.tile([128, NBLK, BLK], BF, name="dvds")

        # phase-1-critical tables load first; the big gather-index tables
        # queue after the xT chunks (they are not needed until SpMM1)
        b.sync.dma_start(W1_s[:, :, :], W1_d.ap().rearrange("(k p) h -> p k h", p=128))
        b.sync.dma_start(dvo_s[:, :], dvo_d.ap())

        def load_late_tables():
            b.sync.dma_start(dl_s[:, :], dl_d.ap())
            b.sync.dma_start(io_s[:, :, :],
                             io_d.ap().rearrange("p (t k) -> p t k", k=BLK))
            for s in range(S):
                b.sync.dma_start(ix_s[s][:, :], ix_d[s].ap())
            b.sync.dma_start(W2_s[:, :, :],
                             W2_d.ap().rearrange("(k p) h -> p k h", p=128))
            b.sync.dma_start(Wc_s[:, :], Wc_d.ap())
            b.sync.dma_start(b1_s[:, :], b1_d.ap())
            b.sync.dma_start(b2_s[:, :], b2_d.ap())
            b.sync.dma_start(bc_s[:, :], bc_d.ap())
            b.sync.dma_start(dvd_s[:, :, :],
                             dvd_d.ap().rearrange("p (c k) -> p c k", k=BLK))

        # DRAM staging (persist; declared first so SBUF frees stay LIFO).
        dshp = tc.alloc_tile_pool(name="dsh", bufs=1, space="DRAM")
        agin = [dshp.tile([RPC[s], HID], FP8, name=f"agin{s}") for s in range(S)]
        hS = [dshp.tile([RTOT[s], HID], FP8, name=f"h{s}", addr_space="Shared")
              for s in range(S)]
        azin = [dshp.tile([RPC[s], MID], BF, name=f"azin{s}") for s in range(S)]
        zS = [dshp.tile([RTOT[s], MID], BF, name=f"z{s}", addr_space="Shared")
              for s in range(S)]

        def stage_group(dst_list, src3, b0, b1):
            """Stage blocks [b0, b1) (node-major SBUF [128, blk, F]) into
            their stream's DRAM staging buffer with one DMA (plus one for a
            trailing partial block). The group must lie within one stream."""
            s = next(i for i in range(S)
                     if BOUNDS[i] <= b0 < (BOUNDS[i + 1] if i < S - 1 else NB128))
            r0 = (b0 - BOUNDS[s]) * 128
            bf = min(b1, NFULL)
            if bf > b0:
                b.sync.dma_start(
                    dst_list[s][r0:r0 + (bf - b0) * 128, :]
                    .rearrange("(c p) f -> p c f", p=128),
                    src3[:, b0:bf, :])
            if b1 > NFULL and NREM:
                rr = (NFULL - BOUNDS[s]) * 128
                b.sync.dma_start(dst_list[s][rr:rr + NREM, :],
                                 src3[:NREM, NFULL, :])

        def ag_fire(which, in_list, out_list):
            b.gpsimd.collective_compute(
                "AllGather", mybir.AluOpType.bypass,
                replica_groups=[list(range(ncores))],
                ins=[in_list[which][:, :].opt()],
                outs=[out_list[which][:, :].opt()])

        # ---- phase 1: h = dinv * (x @ W1), cast fp8 on Scalar, staged and
        # AllGathered per stream chunk as soon as each chunk completes
        hbf, hbf_free = tc.tile([128, NB128, HID], FP8, name="hbf")
        fire_at = {BOUNDS[s + 1] if s < S - 1 else NB128: s for s in range(S)}
        with (tc.tile_pool(name="xtp", bufs=2) as xtp,
              tc.tile_pool(name="ps1p", bufs=3, space="PSUM") as ps1p):
            XCH = 7                      # node-block chunks of the xT load
            for c0 in range(0, NB128, XCH):
                c1 = min(c0 + XCH, NB128)
                xt = xtp.tile([128, IN // 128, XCH * 128], BF, name="xts")
                b.sync.dma_start(
                    xt[:, :, :(c1 - c0) * 128],
                    xT_d.ap().rearrange("(k p) n -> p k n", p=128)
                    [:, :, c0 * 128:c1 * 128])
                for blk in range(c0, c1):
                    ps1 = ps1p.tile([128, HID], F32, name="ps1")
                    for k in range(IN // 128):
                        b.tensor.matmul(
                            ps1[:, :],
                            lhsT=xt[:, k, (blk - c0) * 128:(blk - c0 + 1) * 128],
                            rhs=W1_s[:, k, :],
                            start=(k == 0), stop=(k == IN // 128 - 1))
                    b.scalar.activation(
                        out=hbf[:, blk, :], in_=ps1[:, :],
                        func=mybir.ActivationFunctionType.Copy,
                        scale=dvo_s[:, blk:blk + 1])
                stage_group(agin, hbf, c0, c1)
                for bound, s_id in fire_at.items():
                    if c0 < bound <= c1:
                        ag_fire(s_id, agin, hS)
        hbf_free()
        load_late_tables()

        # ---- SpMM machinery
        qctr = [0]

        def spmm(feats, F, bias_s, layer, fdt, nreg, flush_hook=None):
            """Aggregate feats (list of S DRAM tables [*, F]) per 128-dst
            block: hp = Relu(psum * dinv_dst + bias) (features on partitions)
            handed to flush_hook(blk, hp)."""
            nfc = F // 128
            hpool = tc.alloc_tile_pool(name=f"h{layer}", bufs=4)
            CT = 8                      # tiles per gather call (1024 idx max)
            streams = {
                s: dict(src=feats[s][:, :], idx=ix_s[s], sched=sched[s])
                for s in range(S)
            }
            gmap = {s: {} for s in range(S)}
            gcnt = {s: 0 for s in range(S)}
            gpool = tc.alloc_tile_pool(name=f"g{layer}", bufs=16)
            mpool = tc.alloc_tile_pool(name=f"m{layer}",
                                       bufs=9 if F > 128 else 5)
            pspool = tc.alloc_tile_pool(name=f"ps{layer}",
                                        bufs=3 if F > 128 else 4, space="PSUM")
            sbpool = tc.alloc_tile_pool(name=f"sb{layer}", bufs=4)

            def ensure_gather(s, t):
                st = streams[s]
                while t >= gcnt[s] * CT:
                    c0 = gcnt[s] * CT
                    gt = gpool.tile([128, CT, F], fdt, name=f"gt{layer}{s}")
                    b.gpsimd.dma_gather(
                        out_ap=gt[:, :, :],
                        in_ap=st['src'],
                        idxs_ap=st['idx'][:, c0 * 8:(c0 + CT) * 8],
                        num_idxs=CT * TILE,
                        num_idxs_reg=nreg,
                        elem_size=F,
                        single_packet=True,
                        queue_num=qctr[0] % 4,
                    )
                    qctr[0] += 1
                    gmap[s][gcnt[s]] = gt
                    gcnt[s] += 1

            # pre-issue early-stream gather calls so the in-order gpsimd
            # queue isn't head-of-line blocked by a later stream's AllGather
            # while earlier streams' data is already available
            ensure_gather(0, min(7 * CT, max(sum(sched[0]) - 1, 0)))
            ensure_gather(1, min(3 * CT, max(sum(sched[1]) - 1, 0)))

            cons = {s: 0 for s in range(S)}
            tpos = 0                     # global tile index (consumption order)
            for blk in range(NBLK):
                ntb = sum(streams[s]['sched'][blk] for s in range(S))
                # on-chip one-hot for this block's tiles: one DVE op
                mt = mpool.tile([128, MAXBT, BLK], fdt, name=f"mt{layer}")
                b.vector.tensor_tensor(
                    out=mt[:, :ntb, :],
                    in0=dl_s[:, tpos:tpos + ntb].to_broadcast([128, ntb, BLK]),
                    in1=io_s[:, :ntb, :],
                    op=mybir.AluOpType.is_equal)

                ps = [pspool.tile([128, BLK], F32, name=f"ps{layer}_{fc}")
                      for fc in range(nfc)]
                ti = 0
                for s in range(S):
                    for _ in range(streams[s]['sched'][blk]):
                        t = cons[s]
                        ensure_gather(s, t)
                        gt = gmap[s][t // CT]
                        gc = t % CT
                        for fc in range(nfc):
                            b.tensor.matmul(
                                ps[fc][:, :],
                                lhsT=gt[:, gc, fc * 128:(fc + 1) * 128],
                                rhs=mt[:, ti, :],
                                start=(ti == 0), stop=(ti == ntb - 1))
                        cons[s] += 1
                        ti += 1
                hp = hpool.tile([128, nfc, BLK], BF, name=f"hp{layer}")
                for fc in range(nfc):
                    tmp = sbpool.tile([128, BLK], BF, name=f"tp{layer}_{fc}")
                    b.vector.tensor_tensor(
                        out=tmp[:, :], in0=ps[fc][:, :], in1=dvd_s[:, blk, :],
                        op=mybir.AluOpType.mult)
                    if layer == 2:
                        # DVE relu via two tensor_tensor ops (tensor_scalar
                        # fused/imm forms measured 10x slower on HW): keeps
                        # the Scalar engine exp-only in layer 2 so its
                        # activation table never reloads mid-loop
                        tmp2 = sbpool.tile([128, BLK], BF, name=f"tq{layer}_{fc}")
                        b.vector.tensor_tensor(
                            out=tmp2[:, :], in0=tmp[:, :],
                            in1=bias_s[:, fc:fc + 1].to_broadcast([128, BLK]),
                            op=mybir.AluOpType.add)
                        b.vector.tensor_tensor(
                            out=hp[:, fc, :], in0=tmp2[:, :],
                            in1=zero_s[:, 0:1].to_broadcast([128, BLK]),
                            op=mybir.AluOpType.max)
                    else:
                        b.scalar.activation(
                            out=hp[:, fc, :], in_=tmp[:, :],
                            func=mybir.ActivationFunctionType.Relu,
                            bias=bias_s[:, fc:fc + 1])
                flush_hook(blk, hp)
                tpos += ntb
            for pp in (sbpool, pspool, mpool, gpool, hpool):
                pp.release()

        zero_s, zero_s_free = tc.tile([128, 2], BF, name="zeros")
        b.vector.memset(zero_s[:, :], 0.0)

        # ---- layer 1 SpMM; z2 = dinv * (h1 @ W2) fused per block (Scalar
        # Copy applies the own-node dinv), staged per block, AllGathered in
        # S chunks (first chunks fire mid-SpMM1)
        z2bf, z2bf_free = tc.tile([128, NB128, MID], BF, name="z2bf")
        ps2p = tc.alloc_tile_pool(name="ps2p", bufs=2, space="PSUM")

        def z2hook(blk, hp):
            psz = ps2p.tile([128, MID], F32, name="psz")
            for fc in range(HID // 128):
                b.tensor.matmul(
                    psz[:, :],
                    lhsT=hp[:, fc, :],
                    rhs=W2_s[:, fc, :],
                    start=(fc == 0), stop=(fc == HID // 128 - 1))
            b.scalar.activation(
                out=z2bf[:, blk, :], in_=psz[:, :],
                func=mybir.ActivationFunctionType.Copy,
                scale=dvo_s[:, blk:blk + 1])
            if (blk + 1) % 7 == 0 or blk + 1 == NB128:
                g0 = (blk // 7) * 7
                stage_group(azin, z2bf, g0, blk + 1)
            if blk + 1 in fire_at:
                ag_fire(fire_at[blk + 1], azin, zS)

        spmm(hS, HID, b1_s, layer=1, fdt=FP8, nreg=nreg, flush_hook=z2hook)
        ps2p.release()
        z2bf_free()

        # ---- layer 2 SpMM with fused per-block classifier. Per block: logits
        # + exp + rowsum only (Scalar stays on the Exp table); the Ln +
        # subtract + output DMA run batched after the SpMM (one table load).
        clpool = tc.alloc_tile_pool(name="clp", bufs=4)
        pslp = tc.alloc_tile_pool(name="pslp", bufs=2, space="PSUM")
        ls_s, ls_s_free = tc.tile([128, NBLK, OUT], F32, name="lsall")
        es_s, es_s_free = tc.tile([128, NBLK], F32, name="esall")
        ln_s, ln_s_free = tc.tile([128, NBLK], F32, name="lnall")

        def classify(blk, hp):
            psl = pslp.tile([128, OUT], F32, name="psl")
            b.tensor.matmul(
                psl[:, :], lhsT=hp[:, 0, :],
                rhs=Wc_s[:, :], start=True, stop=True)
            ex = clpool.tile([128, OUT], F32, name="exs")
            b.vector.tensor_tensor(out=ls_s[:, blk, :], in0=psl[:, :],
                                   in1=bc_s[:, :], op=mybir.AluOpType.add)
            b.scalar.activation(out=ex[:, :], in_=ls_s[:, blk, :],
                                func=mybir.ActivationFunctionType.Exp)
            b.vector.reduce_sum(out=es_s[:, blk:blk + 1], in_=ex[:, :],
                                axis=mybir.AxisListType.X)

        spmm(zS, MID, b2_s, layer=2, fdt=BF, nreg=nreg, flush_hook=classify)

        b.scalar.activation(out=ln_s[:, :], in_=es_s[:, :],
                            func=mybir.ActivationFunctionType.Ln)
        # per-block subtract into one tile, then two output DMAs
        ota, ota_free = tc.tile([128, NBLK, OUT], F32, name="otall")
        for blk in range(NBLK):
            b.vector.tensor_tensor(
                out=ota[:, blk, :], in0=ls_s[:, blk, :],
                in1=ln_s[:, blk:blk + 1].to_broadcast([128, OUT]),
                op=mybir.AluOpType.subtract)
        b.sync.dma_start(
            out_d.ap()[:NFULL * 128, :].rearrange("(c p) f -> p c f", p=128),
            ota[:, :NFULL, :])
        if NREM:
            b.sync.dma_start(out_d.ap()[NFULL * 128:, :], ota[:NREM, NFULL, :])

        ota_free()
        ln_s_free(); es_s_free(); ls_s_free()
        pslp.release()
        clpool.release()
        zero_s_free()
        dshp.release()
        dvd_s_free(); dvo_s_free()
        io_s_free(); dl_s_free()
        for f in reversed(ix_free):
            f()
        bc_s_free(); b2_s_free(); b1_s_free()
        Wc_s_free(); W2_s_free(); W1_s_free()

    return b


# ---------------------------------------------------------------- entry point

def make_in_maps(inputs, p, n=None, ncores=NC):
    if n is None:
        n = N
    d = p['d']
    NPC, NB128 = d['NPC'], d['NB128']
    NPAD = NB128 * 128
    NBLK = NB128
    S = len(d['RPC'])
    MAXBT = max(sum(p['sched'][s][b] for s in range(S)) for b in range(NBLK))
    x = np.asarray(inputs['x'], np.float32)
    W1 = np.asarray(inputs['W1'], np.float32).astype(BF16NP)
    W2 = np.asarray(inputs['W2'], np.float32).astype(BF16NP)
    Wc = np.asarray(inputs['Wc'], np.float32).astype(BF16NP)
    b1 = np.asarray(inputs['b1'], np.float32)
    b2 = np.asarray(inputs['b2'], np.float32)
    bc = np.asarray(inputs['bc'], np.float32)
    dinv = p['dinv']

    b1c = b1.reshape(HID // 128, 128).T.copy()
    b2c = b2.reshape(MID // 128, 128).T.copy()
    bcr = np.tile(bc[None, :], (128, 1))
    iota = np.tile(np.arange(BLK, dtype=np.float32)[None, :],
                   (128, MAXBT)).astype(BF16NP)

    nodeof = p['nodeof']
    in_maps = []
    for c in range(ncores):
        rows = nodeof[c * NPC:(c + 1) * NPC]
        xT = np.zeros((IN, NPAD), BF16NP)
        xT[:, :NPC] = x[rows].T.astype(BF16NP)
        dv = dinv[rows]
        tmpv = np.zeros(NB128 * 128, np.float32)
        tmpv[:NPC] = dv
        dvo = tmpv.reshape(NB128, 128).T.copy()      # dinv by (row%128, row//128)
        dvd = np.zeros((128, NBLK * BLK), BF16NP)    # dinv[dst] replicated /parts
        dvd[:, :NPC] = np.tile(dv.astype(BF16NP)[None, :], (128, 1))
        pc = p['per_core'][c]
        im = dict(
            xT=xT, W1=W1, W2=W2, Wc=Wc, b1c=b1c, b2c=b2c, bcr=bcr,
            dl=pc['dl'], iota=iota, dvo=dvo, dvd=dvd)
        for s in range(S):
            im[f'idx{s}'] = pc['idx'][s]
        in_maps.append(im)
    return in_maps


def kernel_with_results(inputs, trace=False, **kw):
    from concourse import bacc
    from concourse import bass_utils

    edge_index = np.asarray(inputs['edge_index'])
    p = plan(edge_index, n=N)
    nc_obj = bacc.Bacc("TRN2", target_bir_lowering=False, debug=False,
                       num_devices=NC, num_swdge_queues=4,
                       dynamic_dma_scratch_size=32768)
    build(nc_obj, p, n=N)
    nc_obj.compile()
    in_maps = make_in_maps(inputs, p, n=N)
    res = bass_utils.run_bass_kernel_spmd(nc_obj, in_maps,
                                          core_ids=list(range(NC)),
                                          trace=trace, **kw)
    out = np.concatenate([np.asarray(res.results[c]['out']) for c in range(NC)],
                         axis=0)
    full = np.empty_like(out)
    full[p['nodeof']] = out
    return full.astype(np.float32), res


def kernel(**inputs) -> np.ndarray:
    return kernel_with_results(inputs)[0]


if __name__ == '__main__':
    import reference
    inputs = {k: np.asarray(v) for k, v in reference.setup_inputs().items()}
    out = kernel(**inputs)
    print('kernel out', out.shape, out.dtype)
